# revision 1
# baseline (speedup 1.0000x reference)
"""COLoRALinear fused kernel for 8 trn2 NeuronCores (Bass/Tile).

Problem: out = x@W.T + b + cw*2*(x@sA.T)@sB.T + (1-cw)*2*sum_t r[b,t]*(x@tA[t].T)@tB[t].T
with routing r = softmax(mean_s(x) @ emb.T), cw = sigmoid(collab_weight).

Sharding: core i -> batch element p=i//2 (2048 tokens), DOUT half h=i%2
(2048 cols). Each core holds its full batch element, so routing is local;
no collectives.

Device plan per core:
  - preload x^T as bf16, SBUF-resident ([128, 32, 2048], 128KB/part)
  - phase A: hid^T[80, 2048] = A_cat @ x^T  (A_cat rows: 8 shared + 64 task
    + 8 task_emb), evict rows 0..71 to bf16, free-reduce rows 72..79 into
    routing logits
  - routing: softmax on one partition, build svec[73] (cw2 / routing-scaled
    / 1.0-for-bias), scale B_cat rows -> bf16
  - main loop: 16 dout-tiles of 128; W^T d-tile streamed fp32->bf16 and
    used as the stationary operand, x^T slices as the moving operand
    (N=512); per d-tile: 4 PSUM banks accumulate 32 k matmuls + 1 LoRA
    down-proj matmul each; evict fp32; store [dout, tok] (host transposes).

Measured (reps-in-NEFF diff): ~408 us/core-iteration, rel err 1.65e-3.
W is staged per k-tile (64KB DMAs) in a dedicated 4-buf pool so the first
d-tiles' matmul chains chase the x preload stream instead of waiting for
it. Deeper W prefetch (wch bufs=3: 574 us) regressed — early W DMAs steal
HBM bandwidth from the startup-critical x preload.
"""
import numpy as np
from contextlib import ExitStack

import concourse.bass as bass
import concourse.tile as tile
from concourse import mybir
from concourse.bass_utils import run_bass_kernel_spmd
from concourse.vector_clock import ScopedClock

B, S, DIN, DOUT, R, T = 4, 2048, 4096, 4096, 8, 8
SCALING = 2.0
N_CORES = 8
P = 128
KT = DIN // P            # 32 k-tiles
S_CORE = S               # tokens per core (one batch element)
N_CORE = DOUT // 2       # dout columns per core
NCH = 256                # n-chunk width
NNC = N_CORE // NCH      # 8 n-chunks
MT = S_CORE // P         # 16 m-tiles
AROWS = 80               # 8 shared + 64 task + 8 emb rows in A_cat
HID = 73                 # 72 lora rows + ones(bias) row
F32 = mybir.dt.float32
BF16 = mybir.dt.bfloat16


class _DrainSplitTileContext(tile.TileContext):
    """Walrus in this container rejects a Drain carrying >1 sem wait (the
    CTRL_NO encoding has one TPB_EVENTS wait slot). Split the exit drain's
    waits across a chain of single-wait drains."""

    def _drain_and_barrier(self, tick_clock, wait_clock):
        drain_inst = self.nc.sync.drain()
        wait_clock.add_sem_waits(
            drain_inst.ins, ScopedClock({None: tick_clock.global_clock})
        )
        si = drain_inst.ins.sync_info
        if si is not None and len(si.on_wait) > 1:
            waits = list(si.on_wait)
            drain_inst.ins.sync_info = mybir.SyncInfo(
                on_wait=[waits[0]], on_update=list(si.on_update)
            )
            for w in waits[1:]:
                extra = self.nc.sync.drain()
                extra.ins.sync_info = mybir.SyncInfo(on_wait=[w], on_update=[])

        self.nc.all_engine_barrier()
        assert self.sems is not None
        popped = self.nc._tile_sem_poison_stack.pop()
        assert popped is self._sem_poison
        self.nc.clear_and_free_semaphores(list(self.sems.allocated().values()))
        self.nc.all_engine_barrier()


_wsplit_counter = [0]


def _split_multi_waits(nc):
    """Walrus here lowers DMA/CTRL instructions with a single TPB_EVENTS wait
    slot and rejects >1 sem wait. Hoist extra waits onto same-engine NoOps
    inserted immediately before the offending instruction (engine program
    order makes this semantics-preserving)."""
    for f in nc.m.functions:
        for blk in f.blocks:
            insts = blk.instructions
            out = []
            changed = False
            for inst in insts:
                si = inst.sync_info
                if si is not None and len(si.on_wait) > 1:
                    waits = list(si.on_wait)
                    for w in waits[:-1]:
                        _wsplit_counter[0] += 1
                        nop = mybir.InstNoOp(name=f"I-wsplit-{_wsplit_counter[0]}")
                        nop.engine = inst.engine
                        nop.sync_info = mybir.SyncInfo(on_wait=[w], on_update=[])
                        out.append(nop)
                    inst.sync_info = mybir.SyncInfo(
                        on_wait=[waits[-1]], on_update=list(si.on_update)
                    )
                    changed = True
                out.append(inst)
            if changed:
                blk.instructions = out


def build_nc(reps: int = 1):
    nc = bass.Bass(trn_type="TRN2", target_bir_lowering=False)
    xt = nc.dram_tensor("xt", [DIN, S_CORE], F32, kind="ExternalInput").ap()
    wt = nc.dram_tensor("wt", [DIN, N_CORE], F32, kind="ExternalInput").ap()
    act = nc.dram_tensor("act", [KT, P, AROWS], F32, kind="ExternalInput").ap()
    bcat = nc.dram_tensor("bcat", [HID, N_CORE], F32, kind="ExternalInput").ap()
    cw = nc.dram_tensor("cw", [1, 1], F32, kind="ExternalInput").ap()
    # output stored [dout, tok]; host assembly transposes back
    out = nc.dram_tensor("out", [N_CORE, S_CORE], F32, kind="ExternalOutput").ap()

    xt_r = xt.rearrange("(kt p) t -> p kt t", p=P)
    wt_r = wt.rearrange("(kt p) n -> p kt n", p=P)

    with _DrainSplitTileContext(nc) as tc, ExitStack() as ctx:
        xres_p = ctx.enter_context(tc.tile_pool(name="xres", bufs=1))
        wch_p = ctx.enter_context(tc.tile_pool(name="wch", bufs=2))
        stage_p = ctx.enter_context(tc.tile_pool(name="stage", bufs=2))
        wstage_p = ctx.enter_context(tc.tile_pool(name="wstage", bufs=4))
        abf_p = ctx.enter_context(tc.tile_pool(name="abf", bufs=1))
        small_p = ctx.enter_context(tc.tile_pool(name="small", bufs=1))
        evict_p = ctx.enter_context(tc.tile_pool(name="evict", bufs=3))
        psb_p = ctx.enter_context(tc.tile_pool(name="psb", bufs=7, space="PSUM"))
        pss_p = ctx.enter_context(tc.tile_pool(name="pss", bufs=1, space="PSUM"))

        for _rep in range(reps):
            # ---- constants / small preloads ----
            a_bf = abf_p.tile([P, KT, AROWS], BF16)
            act_r = act.rearrange("kt p c -> p kt c")
            for half in range(2):
                a_st = stage_p.tile([P, KT // 2, AROWS], F32, tag="stage")
                ks = slice(half * KT // 2, (half + 1) * KT // 2)
                nc.sync.dma_start(out=a_st[:], in_=act_r[:, ks, :])
                nc.vector.tensor_copy(out=a_bf[:, ks, :], in_=a_st[:])

            bmat = small_p.tile([HID, N_CORE], F32)
            nc.sync.dma_start(out=bmat[:], in_=bcat)

            cwt = small_p.tile([1, 1], F32)
            nc.sync.dma_start(out=cwt[:], in_=cw)
            sig = small_p.tile([1, 1], F32)
            nc.scalar.activation(
                out=sig[:], in_=cwt[:], func=mybir.ActivationFunctionType.Sigmoid
            )
            cw2 = small_p.tile([1, 1], F32)
            nc.vector.tensor_scalar_mul(cw2[:], sig[:], SCALING)
            tsc = small_p.tile([1, 1], F32)  # (1 - sigmoid) * SCALING
            nc.vector.tensor_scalar(
                out=tsc[:], in0=sig[:], scalar1=-SCALING, scalar2=SCALING,
                op0=mybir.AluOpType.mult, op1=mybir.AluOpType.add,
            )

            # ---- x preload (fp32 -> bf16, SBUF resident) ----
            xres = xres_p.tile([P, KT, S_CORE], BF16)
            for kt in range(KT):
                xs = stage_p.tile([P, S_CORE], F32, tag="stage")
                nc.sync.dma_start(out=xs[:], in_=xt_r[:, kt, :])
                nc.vector.tensor_copy(out=xres[:, kt, :], in_=xs[:])

            # ---- phase A: hid^T = A_cat @ x^T ----
            hid = small_p.tile([HID, S_CORE], BF16)
            hacc = small_p.tile([AROWS, 1], F32)     # free-reduced phase-A rows
            hpart = small_p.tile([AROWS, 4], F32)
            for c in range(4):
                ph = pss_p.tile([AROWS, 512], F32, tag="pss")
                for kt in range(KT):
                    nc.tensor.matmul(
                        ph[:], lhsT=a_bf[:, kt, :], rhs=xres[:, kt, c * 512:(c + 1) * 512],
                        start=(kt == 0), stop=(kt == KT - 1),
                    )
                nc.vector.tensor_copy(out=hid[0:72, c * 512:(c + 1) * 512], in_=ph[0:72, :])
                nc.vector.tensor_reduce(
                    out=hpart[:, c:c + 1], in_=ph[:], axis=mybir.AxisListType.X,
                    op=mybir.AluOpType.add,
                )
            ones_s = small_p.tile([1, P], BF16)
            nc.vector.memset(ones_s[:], 1.0)
            for m in range(MT):
                nc.sync.dma_start(out=hid[72:73, m * P:(m + 1) * P], in_=ones_s[:])
            nc.vector.tensor_reduce(
                out=hacc[:], in_=hpart[:], axis=mybir.AxisListType.X,
                op=mybir.AluOpType.add,
            )

            # ---- routing ----
            l_row = small_p.tile([1, 8], F32)
            nc.sync.dma_start(out=l_row[:], in_=hacc[72:80, 0:1])  # partition->free
            e_row = small_p.tile([1, 8], F32)
            nc.scalar.activation(
                out=e_row[:], in_=l_row[:], func=mybir.ActivationFunctionType.Exp,
                scale=1.0 / S,
            )
            ssum = small_p.tile([1, 1], F32)
            nc.vector.tensor_reduce(
                out=ssum[:], in_=e_row[:], axis=mybir.AxisListType.X,
                op=mybir.AluOpType.add,
            )
            rec = small_p.tile([1, 1], F32)
            nc.vector.reciprocal(out=rec[:], in_=ssum[:])
            comb = small_p.tile([1, 1], F32)  # (1/sum) * (1-cw)*SCALING
            nc.vector.tensor_tensor(
                out=comb[:], in0=rec[:], in1=tsc[:], op=mybir.AluOpType.mult
            )
            ones8 = small_p.tile([1, 8], F32)
            nc.vector.memset(ones8[:], 1.0)
            svec_f = small_p.tile([1, HID], F32)
            nc.vector.tensor_scalar(
                out=svec_f[0:1, 0:8], in0=ones8[:], scalar1=cw2[:], scalar2=None,
                op0=mybir.AluOpType.mult,
            )
            for t in range(T):
                nc.vector.tensor_scalar(
                    out=svec_f[0:1, 8 + 8 * t:16 + 8 * t], in0=ones8[:],
                    scalar1=e_row[0:1, t:t + 1], scalar2=comb[:],
                    op0=mybir.AluOpType.mult, op1=mybir.AluOpType.mult,
                )
            nc.vector.memset(svec_f[0:1, 72:73], 1.0)
            svec = small_p.tile([HID, 1], F32)
            nc.sync.dma_start(out=svec[:], in_=svec_f[:])  # free->partition
            bbf = small_p.tile([HID, N_CORE], BF16)
            nc.vector.tensor_scalar(
                out=bbf[:], in0=bmat[:], scalar1=svec[:], scalar2=None,
                op0=mybir.AluOpType.mult,
            )

            # ---- main loop: base matmul + fused down-proj ----
            # W^T d-tile is the stationary operand, x^T the moving one
            # (N=512); PSUM tiles come out [dout, tok].
            TC = 4  # token chunks of 512
            for d in range(N_CORE // P):
                wch = wch_p.tile([P, KT, P], BF16)
                for kt in range(KT):
                    ws = wstage_p.tile([P, P], F32)
                    nc.sync.dma_start(
                        out=ws[:], in_=wt_r[:, kt, d * P:(d + 1) * P]
                    )
                    nc.vector.tensor_copy(out=wch[:, kt, :], in_=ws[:])
                pss = [
                    psb_p.tile([P, 512], F32, tag="ps", name=f"ps_{_rep}_{d}_{i}")
                    for i in range(TC)
                ]
                for kt in range(KT):
                    for tcI in range(TC):
                        nc.tensor.matmul(
                            pss[tcI][:], lhsT=wch[:, kt, :],
                            rhs=xres[:, kt, tcI * 512:(tcI + 1) * 512],
                            start=(kt == 0), stop=False,
                        )
                for tcI in range(TC):
                    nc.tensor.matmul(
                        pss[tcI][:], lhsT=bbf[:, d * P:(d + 1) * P],
                        rhs=hid[:, tcI * 512:(tcI + 1) * 512],
                        start=False, stop=True,
                    )
                    ev = evict_p.tile([P, 512], F32)
                    nc.scalar.activation(
                        out=ev[:], in_=pss[tcI][:],
                        func=mybir.ActivationFunctionType.Copy,
                    )
                    nc.scalar.dma_start(
                        out=out[d * P:(d + 1) * P, tcI * 512:(tcI + 1) * 512],
                        in_=ev[:],
                    )
    _split_multi_waits(nc)
    return nc


def prep_inputs(x, W, b, shared_A, shared_B, task_A, task_B, task_emb, collab_weight):
    """Host-side sharding/layout prep. Pure layout: slice/transpose/concat."""
    x = np.asarray(x, dtype=np.float32)
    W = np.asarray(W, dtype=np.float32)
    b = np.asarray(b, dtype=np.float32)
    a_cat = np.concatenate(
        [np.asarray(shared_A), np.asarray(task_A).reshape(T * R, DIN),
         np.asarray(task_emb)], axis=0
    ).astype(np.float32)                                   # [80, DIN]
    act = np.ascontiguousarray(a_cat.T.reshape(KT, P, AROWS))
    cwv = np.asarray(collab_weight, dtype=np.float32).reshape(1, 1)

    xt = [np.ascontiguousarray(x[p].T) for p in range(B)]  # [DIN, S] each
    wt, bc = [], []
    for h in range(2):
        cols = slice(h * N_CORE, (h + 1) * N_CORE)
        wt.append(np.ascontiguousarray(W[cols, :].T))      # [DIN, N_CORE]
        bcat = np.empty((HID, N_CORE), dtype=np.float32)
        bcat[0:8] = np.asarray(shared_B)[cols, :].T
        bcat[8:72] = np.asarray(task_B)[:, cols, :].transpose(0, 2, 1).reshape(
            T * R, N_CORE
        )
        bcat[72] = b[cols]
        bc.append(bcat)

    in_maps = []
    for i in range(N_CORES):
        p, h = i // 2, i % 2
        in_maps.append(
            {"xt": xt[p], "wt": wt[h], "act": act, "bcat": bc[h], "cw": cwv}
        )
    return in_maps


def assemble(results):
    out = np.empty((B, S, DOUT), dtype=np.float32)
    for i in range(N_CORES):
        p, h = i // 2, i % 2
        out[p, :, h * N_CORE:(h + 1) * N_CORE] = results[i]["out"].T
    return out


_NC_CACHE = None


def kernel(**inputs) -> np.ndarray:
    global _NC_CACHE
    if _NC_CACHE is None:
        _NC_CACHE = build_nc()
    in_maps = prep_inputs(**inputs)
    res = run_bass_kernel_spmd(_NC_CACHE, in_maps, core_ids=list(range(N_CORES)))
    return assemble(res.results)



# revision 11
# speedup vs baseline: 741.9729x; 741.9729x over previous
"""COLoRALinear fused kernel for 8 trn2 NeuronCores (Bass/Tile).

Problem: out = x@W.T + b + cw*2*(x@sA.T)@sB.T + (1-cw)*2*sum_t r[b,t]*(x@tA[t].T)@tB[t].T
with routing r = softmax(mean_s(x) @ emb.T), cw = sigmoid(collab_weight).

Sharding: core i -> batch element p=i//2 (2048 tokens), DOUT half h=i%2
(2048 cols). Each core holds its full batch element, so routing is local;
no collectives.

v2 design (vs 574 us baseline): all inputs are pre-converted to bf16 on
the host (halves input HBM traffic, kills all on-chip fp32->bf16 staging
copies), tokens are processed in two 1024-token halves so the next rep's
x preload overlaps this rep's tail, and the whole kernel is emitted as a
software pipeline that keeps the PE array busy from the first x tile:

  phase 1 (kt 0..31, chasing the h0 x stream): phA h0 chunks c0/c1 +
    base d0/d1 chains + per-kt DVE token-sum of h0
  phase 2 (kt 0..31): phA h1 chunks + base d2 + per-kt DVE token-sum of
    h1 and a 1-row PE matmul accumulating routing logits into the spare
    partitions (72:80) of a phase-A PSUM bank
  routing: softmax on one partition, svec scale of B_cat -> bbf (bf16)
  steps 3..31 ((half,d) pairs): 64 base matmuls per step; the
    down-projection + eviction of step s-2 is emitted at kt==8 of step s
    so PSUM banks recycle without stalling the PE (psd bufs=6 = depth 3).

PE floor: (2048 base + 128 phA + 64 downproj) matmuls x 512 rows
= 1.147M cycles @ 2.4 GHz = 478 us/rep.
"""
import numpy as np
from contextlib import ExitStack

import ml_dtypes

import concourse.bass as bass
import concourse.tile as tile
from concourse import mybir
from concourse.bass_utils import run_bass_kernel_spmd
from concourse.vector_clock import ScopedClock

B, S, DIN, DOUT, R, T = 4, 2048, 4096, 4096, 8, 8
SCALING = 2.0
N_CORES = 8
P = 128
KT = DIN // P            # 32 k-tiles
N_CORE = DOUT // 2       # dout columns per core
ND = N_CORE // P         # 16 d-tiles
SH = S // 2              # 1024 tokens per half
AROWS = 72               # 8 shared + 64 task rows in A_cat
EROWS = 8                # task_emb rows (cols 72:80 of acat)
HID = 73                 # 72 lora rows + ones(bias) row
F32 = mybir.dt.float32
BF16 = mybir.dt.bfloat16
BF = ml_dtypes.bfloat16
NSTEP = 2 * ND           # 32 (half, d) steps per rep


class _DrainSplitTileContext(tile.TileContext):
    """Walrus in this container rejects a Drain carrying >1 sem wait (the
    CTRL_NO encoding has one TPB_EVENTS wait slot). Split the exit drain's
    waits across a chain of single-wait drains."""

    def _drain_and_barrier(self, tick_clock, wait_clock):
        drain_inst = self.nc.sync.drain()
        wait_clock.add_sem_waits(
            drain_inst.ins, ScopedClock({None: tick_clock.global_clock})
        )
        si = drain_inst.ins.sync_info
        if si is not None and len(si.on_wait) > 1:
            waits = list(si.on_wait)
            drain_inst.ins.sync_info = mybir.SyncInfo(
                on_wait=[waits[0]], on_update=list(si.on_update)
            )
            for w in waits[1:]:
                extra = self.nc.sync.drain()
                extra.ins.sync_info = mybir.SyncInfo(on_wait=[w], on_update=[])

        self.nc.all_engine_barrier()
        assert self.sems is not None
        popped = self.nc._tile_sem_poison_stack.pop()
        assert popped is self._sem_poison
        self.nc.clear_and_free_semaphores(list(self.sems.allocated().values()))
        self.nc.all_engine_barrier()


_wsplit_counter = [0]


def _split_multi_waits(nc):
    """Walrus here lowers DMA/CTRL instructions with a single TPB_EVENTS wait
    slot and rejects >1 sem wait. Hoist extra waits onto same-engine NoOps
    inserted immediately before the offending instruction (engine program
    order makes this semantics-preserving)."""
    for f in nc.m.functions:
        for blk in f.blocks:
            insts = blk.instructions
            out = []
            changed = False
            for inst in insts:
                si = inst.sync_info
                if si is not None and len(si.on_wait) > 1:
                    waits = list(si.on_wait)
                    for w in waits[:-1]:
                        _wsplit_counter[0] += 1
                        nop = mybir.InstNoOp(name=f"I-wsplit-{_wsplit_counter[0]}")
                        nop.engine = inst.engine
                        nop.sync_info = mybir.SyncInfo(on_wait=[w], on_update=[])
                        out.append(nop)
                    inst.sync_info = mybir.SyncInfo(
                        on_wait=[waits[-1]], on_update=list(si.on_update)
                    )
                    changed = True
                out.append(inst)
            if changed:
                blk.instructions = out


def build_nc(reps: int = 1, split_waits: bool = True):
    nc = bass.Bass(trn_type="TRN2", target_bir_lowering=False)
    xh = nc.dram_tensor("xh", [P, KT, S], BF16, kind="ExternalInput").ap()
    wt = nc.dram_tensor("wt", [P, ND, KT * P], BF16, kind="ExternalInput").ap()
    acat = nc.dram_tensor("acat", [P, KT, AROWS + EROWS], BF16,
                          kind="ExternalInput").ap()
    bcat = nc.dram_tensor("bcat", [HID, N_CORE], BF16, kind="ExternalInput").ap()
    cw = nc.dram_tensor("cw", [1, 1], F32, kind="ExternalInput").ap()
    # output stored [dout, tok]; host assembly transposes back
    out = nc.dram_tensor("out", [N_CORE, S], F32, kind="ExternalOutput").ap()

    with _DrainSplitTileContext(nc) as tc, ExitStack() as ctx:
        const_p = ctx.enter_context(tc.tile_pool(name="const", bufs=1))
        xres_p = ctx.enter_context(tc.tile_pool(name="xres", bufs=2))
        wch_p = ctx.enter_context(tc.tile_pool(name="wch", bufs=3))
        med_p = ctx.enter_context(tc.tile_pool(name="med", bufs=2))
        ev_p = ctx.enter_context(tc.tile_pool(name="ev", bufs=4))
        psa_p = ctx.enter_context(tc.tile_pool(name="psa", bufs=2, space="PSUM"))
        psd_p = ctx.enter_context(tc.tile_pool(name="psd", bufs=6, space="PSUM"))

        # ---- constants (shared by all reps) ----
        a_bf = const_p.tile([P, KT, AROWS + EROWS], BF16)
        nc.sync.dma_start(out=a_bf[:], in_=acat)
        bmat = const_p.tile([HID, N_CORE], BF16)
        nc.sync.dma_start(out=bmat[:], in_=bcat)
        cwt = const_p.tile([1, 1], F32)
        nc.sync.dma_start(out=cwt[:], in_=cw)
        sig = const_p.tile([1, 1], F32)
        nc.scalar.activation(
            out=sig[:], in_=cwt[:], func=mybir.ActivationFunctionType.Sigmoid
        )
        cw2 = const_p.tile([1, 1], F32)  # sigmoid(cw) * SCALING
        nc.vector.tensor_scalar_mul(cw2[:], sig[:], SCALING)
        tsc = const_p.tile([1, 1], F32)  # (1 - sigmoid) * SCALING
        nc.vector.tensor_scalar(
            out=tsc[:], in0=sig[:], scalar1=-SCALING, scalar2=SCALING,
            op0=mybir.AluOpType.mult, op1=mybir.AluOpType.add,
        )
        ones8 = const_p.tile([1, 8], F32)
        nc.vector.memset(ones8[:], 1.0)
        ones_s = const_p.tile([1, S], BF16)
        nc.vector.memset(ones_s[:], 1.0)

        for _rep in range(reps):
            xr = [
                xres_p.tile([P, KT, SH], BF16, tag="x", name=f"x_{_rep}_{h}")
                for h in range(2)
            ]
            hid = med_p.tile([HID, S], BF16, tag="hid", name=f"hid_{_rep}")
            bbf = med_p.tile([HID, N_CORE], BF16, tag="bbf", name=f"bbf_{_rep}")
            hpart = med_p.tile([80, 4], F32, tag="hp", name=f"hp_{_rep}")

            # x preload: h0 then h1 on the SP DMA queue
            for h in range(2):
                for kt in range(KT):
                    nc.sync.dma_start(
                        out=xr[h][:, kt, :], in_=xh[:, kt, h * SH:(h + 1) * SH]
                    )
            nc.scalar.dma_start(out=hid[HID - 1:HID, :], in_=ones_s[:])  # bias row

            wtiles, ptiles = {}, {}

            def stage_w(s, _rep=_rep, wtiles=wtiles):
                half, d = divmod(s, ND)
                wtl = wch_p.tile([P, KT, P], BF16, tag="w", name=f"w_{_rep}_{s}")
                nc.gpsimd.dma_start(out=wtl[:], in_=wt[:, d, :])
                wtiles[s] = wtl

            def alloc_ps(s, _rep=_rep, ptiles=ptiles):
                ptiles[s] = [
                    psd_p.tile([P, 512], F32, tag="psd", name=f"pd_{_rep}_{s}_{i}")
                    for i in range(2)
                ]

            def base_mm(s, kt):
                half, d = divmod(s, ND)
                for tcI in range(2):
                    nc.tensor.matmul(
                        ptiles[s][tcI][:], lhsT=wtiles[s][:, kt, :],
                        rhs=xr[half][:, kt, tcI * 512:(tcI + 1) * 512],
                        start=(kt == 0), stop=False,
                    )

            def dp_evict(s, _rep=_rep):
                half, d = divmod(s, ND)
                for tcI in range(2):
                    off = half * SH + tcI * 512
                    nc.tensor.matmul(
                        ptiles[s][tcI][:], lhsT=bbf[:, d * P:(d + 1) * P],
                        rhs=hid[:, off:off + 512], start=False, stop=True,
                    )
                    ev = ev_p.tile([P, 512], F32, tag="ev",
                                   name=f"ev_{_rep}_{s}_{tcI}")
                    nc.scalar.activation(
                        out=ev[:], in_=ptiles[s][tcI][:],
                        func=mybir.ActivationFunctionType.Copy,
                    )
                    nc.scalar.dma_start(
                        out=out[d * P:(d + 1) * P, off:off + 512], in_=ev[:]
                    )

            for s in (0, 1, 2):
                stage_w(s)

            # ---- phase 1: phA h0 + base d0,d1 chase the h0 stream ----
            pha = [
                psa_p.tile([80, 512], F32, tag="psa", name=f"pa_{_rep}_0_{c}")
                for c in range(2)
            ]
            alloc_ps(0)
            alloc_ps(1)
            for kt in range(KT):
                for c in range(2):
                    nc.tensor.matmul(
                        pha[c][:], lhsT=a_bf[:, kt, :],
                        rhs=xr[0][:, kt, c * 512:(c + 1) * 512],
                        start=(kt == 0), stop=(kt == KT - 1),
                    )
                base_mm(0, kt)
                base_mm(1, kt)
            for c in range(2):
                nc.vector.tensor_copy(
                    out=hid[0:AROWS, c * 512:(c + 1) * 512], in_=pha[c][0:AROWS, :]
                )
                nc.vector.tensor_reduce(
                    out=hpart[:, c:c + 1], in_=pha[c][:],
                    axis=mybir.AxisListType.X, op=mybir.AluOpType.add,
                )

            # ---- phase 2: phA h1 + base d2; logits ride pha h1 c0's
            # spare partitions 72:80 ----
            phb = [
                psa_p.tile([80, 512], F32, tag="psa", name=f"pa_{_rep}_1_{c}")
                for c in range(2)
            ]
            alloc_ps(2)
            for kt in range(KT):
                for c in range(2):
                    nc.tensor.matmul(
                        phb[c][:], lhsT=a_bf[:, kt, :],
                        rhs=xr[1][:, kt, c * 512:(c + 1) * 512],
                        start=(kt == 0), stop=(kt == KT - 1),
                    )
                base_mm(2, kt)
            for c in range(2):
                nc.vector.tensor_copy(
                    out=hid[0:AROWS, SH + c * 512:SH + (c + 1) * 512],
                    in_=phb[c][0:AROWS, :],
                )
                nc.vector.tensor_reduce(
                    out=hpart[:, 2 + c:3 + c], in_=phb[c][:],
                    axis=mybir.AxisListType.X, op=mybir.AluOpType.add,
                )

            # ---- routing: softmax(logits / S), svec, bbf ----
            hacc = med_p.tile([80, 1], F32, tag="hacc", name=f"hacc_{_rep}")
            nc.vector.tensor_reduce(
                out=hacc[:], in_=hpart[:], axis=mybir.AxisListType.X,
                op=mybir.AluOpType.add,
            )
            lrow = med_p.tile([1, EROWS], F32, tag="lrow", name=f"lrow_{_rep}")
            nc.sync.dma_start(out=lrow[:], in_=hacc[AROWS:AROWS + EROWS, 0:1])
            erow = med_p.tile([1, EROWS], F32, tag="erow", name=f"erow_{_rep}")
            nc.scalar.activation(
                out=erow[:], in_=lrow[:], func=mybir.ActivationFunctionType.Exp,
                scale=1.0 / S,
            )
            ssum = med_p.tile([1, 1], F32, tag="ssum", name=f"ssum_{_rep}")
            nc.vector.tensor_reduce(
                out=ssum[:], in_=erow[:], axis=mybir.AxisListType.X,
                op=mybir.AluOpType.add,
            )
            rec = med_p.tile([1, 1], F32, tag="rec", name=f"rec_{_rep}")
            nc.vector.reciprocal(out=rec[:], in_=ssum[:])
            comb = med_p.tile([1, 1], F32, tag="comb", name=f"comb_{_rep}")
            nc.vector.tensor_tensor(
                out=comb[:], in0=rec[:], in1=tsc[:], op=mybir.AluOpType.mult
            )
            svecf = med_p.tile([1, HID], F32, tag="svecf", name=f"svecf_{_rep}")
            nc.vector.tensor_scalar(
                out=svecf[0:1, 0:8], in0=ones8[:], scalar1=cw2[:], scalar2=None,
                op0=mybir.AluOpType.mult,
            )
            for t in range(T):
                nc.vector.tensor_scalar(
                    out=svecf[0:1, 8 + 8 * t:16 + 8 * t], in0=ones8[:],
                    scalar1=erow[0:1, t:t + 1], scalar2=comb[:],
                    op0=mybir.AluOpType.mult, op1=mybir.AluOpType.mult,
                )
            nc.vector.memset(svecf[0:1, HID - 1:HID], 1.0)
            svec = med_p.tile([HID, 1], F32, tag="svec", name=f"svec_{_rep}")
            nc.sync.dma_start(out=svec[:], in_=svecf[:])  # free->partition
            nc.vector.tensor_scalar(
                out=bbf[:], in0=bmat[:], scalar1=svec[:], scalar2=None,
                op0=mybir.AluOpType.mult,
            )

            # ---- steps 3..31: software-pipelined base + down-proj ----
            dp_evict(0)
            dp_evict(1)
            for s in range(3, NSTEP):
                stage_w(s)
                alloc_ps(s)
                for kt in range(KT):
                    base_mm(s, kt)
                    if kt == 8:
                        dp_evict(s - 1)
            dp_evict(NSTEP - 1)

    if split_waits:
        _split_multi_waits(nc)
    return nc


def prep_inputs(x, W, b, shared_A, shared_B, task_A, task_B, task_emb, collab_weight):
    """Host-side sharding/layout prep: slice/transpose/concat + bf16 cast."""
    x = np.asarray(x, dtype=np.float32)
    W = np.asarray(W, dtype=np.float32)
    b = np.asarray(b, dtype=np.float32)
    acat_rows = np.concatenate(
        [np.asarray(shared_A, np.float32),
         np.asarray(task_A, np.float32).reshape(T * R, DIN),
         np.asarray(task_emb, np.float32)], axis=0
    )                                                    # [80, DIN]
    acat = np.ascontiguousarray(
        acat_rows.T.reshape(KT, P, AROWS + EROWS).transpose(1, 0, 2)
    ).astype(BF)                                         # [P, KT, 80]
    cwv = np.asarray(collab_weight, dtype=np.float32).reshape(1, 1)

    xhs = []
    for p in range(B):
        xt = x[p].T                                      # [DIN, S]
        xhs.append(
            np.ascontiguousarray(
                xt.reshape(KT, P, S).transpose(1, 0, 2)
            ).astype(BF)                                 # [P, KT, S]
        )
    wts, bcs = [], []
    for h in range(2):
        cols = slice(h * N_CORE, (h + 1) * N_CORE)
        w4 = W[cols, :].reshape(ND, P, KT, P)            # [d, n, kt, p]
        wts.append(
            np.ascontiguousarray(w4.transpose(3, 0, 2, 1))
            .reshape(P, ND, KT * P).astype(BF)           # [P, ND, KT*P]
        )
        bcat = np.empty((HID, N_CORE), dtype=np.float32)
        bcat[0:8] = np.asarray(shared_B, np.float32)[cols, :].T
        bcat[8:72] = np.asarray(task_B, np.float32)[:, cols, :].transpose(
            0, 2, 1
        ).reshape(T * R, N_CORE)
        bcat[72] = b[cols]
        bcs.append(bcat.astype(BF))

    in_maps = []
    for i in range(N_CORES):
        p, h = i // 2, i % 2
        in_maps.append(
            {"xh": xhs[p], "wt": wts[h], "acat": acat, "bcat": bcs[h], "cw": cwv}
        )
    return in_maps


def assemble(results):
    out = np.empty((B, S, DOUT), dtype=np.float32)
    for i in range(N_CORES):
        p, h = i // 2, i % 2
        out[p, :, h * N_CORE:(h + 1) * N_CORE] = results[i]["out"].T
    return out


_NC_CACHE = None


def kernel(**inputs) -> np.ndarray:
    global _NC_CACHE
    if _NC_CACHE is None:
        _NC_CACHE = build_nc()
    in_maps = prep_inputs(**inputs)
    res = run_bass_kernel_spmd(_NC_CACHE, in_maps, core_ids=list(range(N_CORES)))
    return assemble(res.results)
